# revision 81
# baseline (speedup 1.0000x reference)
"""MoD router kernel for Trainium2 (Bass/Tile), 8 NeuronCores, batch-parallel.

Problem (per batch b of 8):
    scores = x[b] @ w_router                       # (4096,)
    topk_scores, idx = top_k(scores, 3072)         # sorted desc
    routed = x[b][idx]                             # (3072, 1024)
    w = softmax(topk_scores)[:, None]
    blended = processed[b] * w + (1 - w) * routed
    out[b] = x[b];  out[b][idx] = blended

Rank identity (no sort): position p with rank r_p = #{j : s_j > s_p} is
selected iff r_p < K, blends with processed[r_p] and weight e^{s_p}/Z.

v3 design (cost-model driven, HW-verified engine constraints):
  - x lives in SBUF as bf16 via CASTING gpsimd DMAs (the cost model charges
    converting DMAs by OUTPUT bytes: 8 MiB not 16); out DRAM is bf16 with a
    host upcast (8 MiB stores); proc rows gathered per 8-group chunk in one
    indirect DMA, cast to bf16.
  - PAIRWISE rank counting at HALF the work: the 32x32 grid of 128-block
    comparisons is computed once (upper wedge only). Each row-block a runs
    ONE instruction over columns [128a, 4096) producing the comparison
    matrix M (bf16) plus its row-sum (accum_out). The transposed (lower)
    counts come from PE column-sum matmuls (M.T @ ones) accumulated in
    PSUM — integer-exact in bf16/fp32.
  - Blocks a<8 run on ACT via the Sign trick (M' = Sign(s_i - s_j), split
    into 3 column ranges so ACT starts before all scores exist); blocks
    a>=8 on DVE via is_gt tensor_scalar (2x mode). GPSIMD supports neither
    (HW engine check) and cannot read PSUM; it does the DMAs instead.
  - rank_b (b<8)  = -0.5*sgnrow_b + 0.5*pscol_s[b] + 2047.5
    rank_b (b>=8) = up_b + 0.5*pscol_s[b] - pscol_g[b] + 128b - 512
  - Cost model (HW exec): 112.9us vs the 158.1us v1 baseline.
  - blend: diag(w) @ proc + diag(1-w) @ x on PE into PSUM (bf16 diag
    matmuls), drained to SBUF by ACT/DVE copies; some pairs blended
    directly on DVE. Stores bound the tail.
"""

import numpy as np

import concourse.bacc as bacc
import concourse.bass as bass
import concourse.mybir as mybir
from concourse.bass import IndirectOffsetOnAxis
from concourse.masks import make_identity
from concourse.tile import TileContext

B, S, D, K = 8, 4096, 1024, 3072
P = 128
G = S // P           # 32 position groups of 128
H = D // 2           # blend half width (one PSUM bank of fp32)
FP32 = mybir.dt.float32
BF16 = mybir.dt.bfloat16
I32 = mybir.dt.int32

# --- tunables -----------------------------------------------------------
LOAD_CHUNKS = [2, 2, 4, 4, 4, 4, 4, 4, 4]  # x-load groups per casting DMA
A1 = 8               # blocks owned by ACT (Sign); the rest by DVE (is_gt)
C1, C2 = 2048, 3072  # ACT column-split points (3 sub-ranges)
CHUNK = 8            # groups per rank/gather chunk
NCH = G // CHUNK
DVE_PAIRS = frozenset({5, 7, 9, 11, 13, 15, 1, 3})  # blend pairs on DVE
POOL_PAIRS = frozenset()                        # Pool TT blends: too slow
SBC0 = A1 * P        # first column of the positive score broadcast


def build_nc() -> bass.Bass:
    nc = bacc.Bacc("TRN2", target_bir_lowering=False, num_devices=B)

    x = nc.dram_tensor("x", [S, D], FP32, kind="ExternalInput").ap()
    proc = nc.dram_tensor("proc", [K, D], FP32, kind="ExternalInput").ap()
    w_in = nc.dram_tensor("w", [1, D], FP32, kind="ExternalInput").ap()
    out = nc.dram_tensor("out", [S, D], BF16, kind="ExternalOutput").ap()

    alu = mybir.AluOpType
    act = mybir.ActivationFunctionType
    pt_tiles = {}

    with TileContext(nc) as tc:
        with (
            tc.tile_pool(name="persist", bufs=1) as pp,
            tc.tile_pool(name="scrv", bufs=2) as svp,
            tc.tile_pool(name="ma1", bufs=2) as map1,
            tc.tile_pool(name="ma2", bufs=8) as map2,
            tc.tile_pool(name="ma3", bufs=8) as map3,
            tc.tile_pool(name="md", bufs=2) as mdp,
            tc.tile_pool(name="proctile", bufs=2) as prp,
            tc.tile_pool(name="store", bufs=2) as stp,
            tc.tile_pool(name="diag", bufs=6) as dgp,
            tc.tile_pool(name="psum_t", bufs=2, space="PSUM") as ptp,
            tc.tile_pool(name="psum_b", bufs=5, space="PSUM") as pbp,
            tc.tile_pool(name="psum_z", bufs=1, space="PSUM") as pzp,
        ):
            # ---- persistent tiles ----
            x_sb = pp.tile([P, G, D], BF16)        # 64 KiB/part
            nsbc = pp.tile([P, S], FP32)           # NEGATED score bcast (ACT)
            sbc = pp.tile([P, S - SBC0], FP32)     # +score bcast, cols>=1280
            wbc = pp.tile([P, D], BF16)            # router weights bcast
            ident = pp.tile([P, P], FP32)
            ident16 = pp.tile([P, P], BF16)
            ones16 = pp.tile([1, P], BF16)
            onec16 = pp.tile([P, 1], BF16)         # colsum rhs
            zer128 = pp.tile([P, P], BF16)         # dummy colsum lhsT
            w_sb = pp.tile([1, D], FP32)
            w16 = pp.tile([1, D], BF16)
            s_col = pp.tile([P, G], FP32)          # s[g*128+p] at [p, g]
            sgn1 = pp.tile([P, A1], FP32)          # ACT Sign row-sums (parts)
            sgn2 = pp.tile([P, A1], FP32)
            sgn3 = pp.tile([P, A1], FP32)
            up = pp.tile([P, G], FP32)             # DVE is_gt row-sums
            cfix = pp.tile([P, G], FP32)
            cfix_i = pp.tile([P, G - A1], I32)
            rank = pp.tile([P, G], FP32)
            e_col = pp.tile([P, G], FP32)
            em = pp.tile([P, G], FP32)
            w_col = pp.tile([P, G], FP32)
            omw = pp.tile([P, G], FP32)
            gidx = pp.tile([P, G], I32)
            sg8 = pp.tile([P, A1], FP32)           # scratch sgn sum
            z_part = pp.tile([P, 1], FP32)
            z_all = pp.tile([P, 1], FP32)
            z_inv = pp.tile([P, 1], FP32)
            # shared PSUM bank: cols 0..63 = lower-count accumulators
            # (col b = sign-type, col 32+b = gt-type), cols 64..191 = Z bcast
            zbank = pzp.tile([P, 2 * G + P], FP32, tag="pz")
            pscol = zbank[:, 0:2 * G]

            # ---- constants ----
            make_identity(nc, ident)
            nc.vector.tensor_copy(out=ident16, in_=ident)
            nc.vector.memset(ones16, 1.0)
            nc.vector.memset(onec16, 1.0)
            nc.vector.memset(zer128, 0.0)
            # rank fixups (see module docstring formulas)
            nc.vector.memset(cfix[:, :A1], (S - 1) / 2.0)
            nc.gpsimd.iota(
                cfix_i, pattern=[[P, G - A1]], base=64 * A1,
                channel_multiplier=0,
            )
            nc.vector.tensor_copy(out=cfix[:, A1:], in_=cfix_i)

            # router weights: DMA one row, cast, broadcast via PE
            nc.sync.dma_start(out=w_sb, in_=w_in)
            nc.vector.tensor_copy(out=w16, in_=w_sb)
            for h in range(2):
                pw = ptp.tile([P, H], FP32, tag="pst")
                nc.tensor.matmul(
                    out=pw, lhsT=ones16, rhs=w16[:, h * H:(h + 1) * H],
                    start=True, stop=True,
                )
                nc.scalar.copy(out=wbc[:, h * H:(h + 1) * H], in_=pw)

            # dummy colsum accumulations for columns with no contributors
            nc.tensor.matmul(out=pscol[:, 0:1], lhsT=zer128,
                             rhs=onec16, start=True, stop=True)
            nc.tensor.matmul(out=pscol[:, G + A1:G + A1 + 1],
                             lhsT=zer128, rhs=onec16,
                             start=True, stop=True)

            # ---- x loads (gpsimd casting DMA fp32->bf16) ----
            g0 = 0
            for n in LOAD_CHUNKS:
                src = x[g0 * P:(g0 + n) * P, :].rearrange(
                    "(g p) d -> p g d", p=P
                )
                nc.gpsimd.dma_start(out=x_sb[:, g0:g0 + n, :], in_=src)
                g0 += n

            # ---- scores (DVE) + score broadcasts, in chunks of 4 groups ---
            for c in range(G // 4):
                for k in range(4):
                    g = c * 4 + k
                    scr = svp.tile([P, D], BF16, tag="sv")
                    nc.vector.scalar_tensor_tensor(
                        out=scr, in0=x_sb[:, g, :], scalar=1.0, in1=wbc,
                        op0=alu.bypass, op1=alu.mult,
                        accum_out=s_col[:, g:g + 1],
                    )
                pst = ptp.tile([P, 4 * P], FP32, tag="pst")
                for k in range(4):
                    g = c * 4 + k
                    nc.tensor.transpose(
                        out=pst[:, k * P:(k + 1) * P],
                        in_=s_col[:, g:g + 1].to_broadcast([P, P]),
                        identity=ident,
                    )
                col0, col1 = c * 4 * P, (c + 1) * 4 * P
                # ACT holds the NEGATED broadcast (free via copy scale)
                nc.scalar.mul(out=nsbc[:, col0:col1], in_=pst, mul=-1.0)
                a, b = max(col0, SBC0), col1
                if a < b:
                    # early chunks fit in ACT's idle window; late chunks
                    # would delay ACT's Sign stream, keep them on DVE
                    if c <= 5:
                        nc.scalar.copy(
                            out=sbc[:, a - SBC0:b - SBC0],
                            in_=pst[:, a - col0:b - col0],
                        )
                    else:
                        nc.vector.tensor_copy(
                            out=sbc[:, a - SBC0:b - SBC0],
                            in_=pst[:, a - col0:b - col0],
                        )
                # e = exp(s): |s| < ~4 so exp cannot overflow; the constant
                # shift cancels in w = e/Z.
                nc.scalar.activation(
                    out=e_col[:, c * 4:(c + 1) * 4],
                    in_=s_col[:, c * 4:(c + 1) * 4], func=act.Exp,
                )

            # ---- pairwise counting ----
            # ACT block a < A1: M' = Sign(s_a - s_j) over j in [128a, S),
            # emitted as 3 column sub-ranges so ACT starts early.
            # DVE block a >= A1: M = (s_j > s_a) over j in [128a, S).
            # PE accumulates column sums of every M into pscol.

            def sign_part(a, lo, hi, pool, accum):
                t = pool.tile([P, hi - lo if lo else C1], BF16, tag="m")
                w = hi - max(lo, a * P)
                tv = t[:, :w]
                nc.scalar.activation(
                    out=tv, in_=nsbc[:, max(lo, a * P):hi], func=act.Sign,
                    bias=s_col[:, a:a + 1], accum_out=accum[:, a:a + 1],
                )
                return tv, max(lo, a * P)

            def colsum(m, mbase, a, b, first, last, col):
                off = b * P - mbase
                nc.tensor.matmul(
                    out=pscol[:, col:col + 1], lhsT=m[:, off:off + P],
                    rhs=onec16, start=(a == first), stop=(a == last),
                    skip_group_check=True,
                )

            # ACT part 1: columns [128a, C1), colsums inline (feeds chunk 1).
            # Parts 2/3 run back-to-back on ACT; their colsums are deferred
            # (tiles stay live — 8-deep pools) so PE serves chunk 1 first.
            for a in range(A1):
                m, mb = sign_part(a, 0, C1, map1, sgn1)
                for b in range(a + 1, C1 // P):
                    colsum(m, mb, a, b, 0, min(b, A1) - 1, b)
            m2t = [sign_part(a, C1, C2, map2, sgn2) for a in range(A1)]
            m3t = [sign_part(a, C2, S, map3, sgn3) for a in range(A1)]


            def combine(cc):
                cs = slice(cc * CHUNK, (cc + 1) * CHUNK)
                if cc == 0:
                    nc.vector.tensor_tensor(
                        out=sg8[:, 0:CHUNK], in0=sgn1[:, 0:CHUNK],
                        in1=sgn2[:, 0:CHUNK], op=alu.add,
                    )
                    nc.vector.tensor_tensor(
                        out=sg8[:, 0:CHUNK], in0=sg8[:, 0:CHUNK],
                        in1=sgn3[:, 0:CHUNK], op=alu.add,
                    )
                    nc.vector.scalar_tensor_tensor(
                        out=rank[:, cs], in0=sg8[:, 0:CHUNK], scalar=-0.5,
                        in1=cfix[:, cs], op0=alu.mult, op1=alu.add,
                    )
                    nc.vector.scalar_tensor_tensor(
                        out=rank[:, cs], in0=pscol[:, cs], scalar=0.5,
                        in1=rank[:, cs], op0=alu.mult, op1=alu.add,
                    )
                else:
                    nc.vector.scalar_tensor_tensor(
                        out=rank[:, cs], in0=pscol[:, G + cc * CHUNK:
                                                   G + (cc + 1) * CHUNK],
                        scalar=-1.0, in1=up[:, cs],
                        op0=alu.mult, op1=alu.add,
                    )
                    nc.vector.scalar_tensor_tensor(
                        out=rank[:, cs], in0=pscol[:, cs], scalar=0.5,
                        in1=rank[:, cs], op0=alu.mult, op1=alu.add,
                    )
                    nc.vector.tensor_tensor(
                        out=rank[:, cs], in0=rank[:, cs], in1=cfix[:, cs],
                        op=alu.add,
                    )
                nc.vector.tensor_scalar(
                    out=gidx[:, cs], in0=rank[:, cs], scalar1=float(K - 1),
                    scalar2=None, op0=alu.min,
                )
                # em = (rank < K) * e
                nc.vector.scalar_tensor_tensor(
                    out=em[:, cs], in0=rank[:, cs], scalar=float(K),
                    in1=e_col[:, cs], op0=alu.is_lt, op1=alu.mult,
                )

            def gather(cc):
                pt = prp.tile([P, CHUNK, D], BF16, tag="pt")
                for j in range(CHUNK):
                    g = cc * CHUNK + j
                    nc.gpsimd.indirect_dma_start(
                        out=pt[:, j, :], out_offset=None, in_=proc,
                        in_offset=IndirectOffsetOnAxis(
                            ap=gidx[:, g:g + 1], axis=0
                        ),
                    )
                pt_tiles[cc] = pt

            # DVE upper blocks + their gt-type colsums. Manual logical
            # timestamps (tile_wait_until) force the scheduler's frozen
            # per-engine order to interleave combines/gathers right after
            # their true producers — its internal cost model would
            # otherwise schedule them after more counting, idling the DMA
            # engines for ~35us. The waits only shape the static order;
            # the real sync is still semaphores.
            def upper(a):
                L = S - a * P
                m = mdp.tile([P, S - A1 * P], BF16, tag="m")
                nc.vector.tensor_scalar(
                    out=m[:, :L], in0=sbc[:, a * P - SBC0:],
                    scalar1=s_col[:, a:a + 1], scalar2=None,
                    op0=alu.is_gt, op1=alu.add, accum_out=up[:, a:a + 1],
                )
                for b in range(a + 1, G):
                    colsum(m, a * P, a, b, A1, b - 1, G + b)

            for a in range(A1, 16):
                upper(a)
            with tc.tile_wait_until(0.066):
                combine(1)
                gather(1)
            with tc.tile_wait_until(0.068):
                for a in range(16, 24):
                    upper(a)
            with tc.tile_wait_until(0.070):
                for a in range(A1):
                    m, mb = m2t[a]
                    for b in range(C1 // P, C2 // P):
                        colsum(m, mb, a, b, 0, A1 - 1, b)
            with tc.tile_wait_until(0.072):
                combine(2)
                gather(2)
            with tc.tile_wait_until(0.074):
                for a in range(24, G):
                    upper(a)
            with tc.tile_wait_until(0.076):
                for a in range(A1):
                    m, mb = m3t[a]
                    for b in range(C2 // P, G):
                        colsum(m, mb, a, b, 0, A1 - 1, b)
            with tc.tile_wait_until(0.078):
                combine(3)
                gather(3)
                combine(0)
                gather(0)

            # ---- Z and weights ----
            nc.vector.tensor_reduce(
                out=z_part, in_=em, axis=mybir.AxisListType.X, op=alu.add
            )
            pz = zbank[:, 2 * G:2 * G + P]
            nc.tensor.transpose(
                out=pz, in_=z_part[:, 0:1].to_broadcast([P, P]), identity=ident
            )
            nc.vector.tensor_reduce(
                out=z_all, in_=pz, axis=mybir.AxisListType.X, op=alu.add
            )
            nc.vector.reciprocal(out=z_inv, in_=z_all)
            nc.vector.tensor_scalar(
                out=w_col, in0=em, scalar1=z_inv[:, 0:1], scalar2=None,
                op0=alu.mult,
            )
            nc.vector.tensor_scalar(
                out=omw, in0=w_col, scalar1=-1.0, scalar2=1.0,
                op0=alu.mult, op1=alu.add,
            )

            # ---- blend + store ---- (chunk 0's ranks arrive last)
            ncopy = 0
            for p2 in list(range(4, G // 2)) + list(range(4)):
                ga = 2 * p2
                cc, j0 = divmod(ga, CHUNK)
                pt = pt_tiles[cc]
                if p2 in POOL_PAIRS:
                    # Pool blends via TT with per-partition broadcasts
                    # (gpsimd has no tensor_scalar on HW)
                    for j in (0, 1):
                        g = ga + j
                        ptg = pt[:, j0 + j, :]
                        nc.gpsimd.tensor_tensor(
                            out=ptg, in0=ptg,
                            in1=w_col[:, g:g + 1].to_broadcast([P, D]),
                            op=alu.mult,
                        )
                        nc.gpsimd.tensor_tensor(
                            out=x_sb[:, g, :], in0=x_sb[:, g, :],
                            in1=omw[:, g:g + 1].to_broadcast([P, D]),
                            op=alu.mult,
                        )
                        nc.gpsimd.tensor_tensor(
                            out=x_sb[:, g, :], in0=x_sb[:, g, :], in1=ptg,
                            op=alu.add,
                        )
                    src2 = x_sb[:, ga:ga + 2, :]
                    dst2 = out[ga * P:(ga + 2) * P, :].rearrange(
                        "(g p) d -> p g d", p=P
                    )
                    nc.sync.dma_start(out=dst2, in_=src2)
                    continue
                if p2 in DVE_PAIRS:
                    for j in (0, 1):
                        g = ga + j
                        ptg = pt[:, j0 + j, :]
                        nc.vector.tensor_scalar(
                            out=ptg, in0=ptg, scalar1=w_col[:, g:g + 1],
                            scalar2=None, op0=alu.mult,
                        )
                        nc.vector.tensor_scalar(
                            out=x_sb[:, g, :], in0=x_sb[:, g, :],
                            scalar1=omw[:, g:g + 1], scalar2=None,
                            op0=alu.mult,
                        )
                        nc.vector.tensor_tensor(
                            out=x_sb[:, g, :], in0=x_sb[:, g, :], in1=ptg,
                            op=alu.add,
                        )
                    src = x_sb[:, ga:ga + 2, :]
                else:
                    st = stp.tile([P, 2, D], BF16, tag="st")
                    for j in (0, 1):
                        g = ga + j
                        dw = dgp.tile([P, P], BF16, tag="dg")
                        do = dgp.tile([P, P], BF16, tag="dg")
                        nc.vector.tensor_scalar(
                            out=dw, in0=ident16, scalar1=w_col[:, g:g + 1],
                            scalar2=None, op0=alu.mult,
                        )
                        nc.vector.tensor_scalar(
                            out=do, in0=ident16, scalar1=omw[:, g:g + 1],
                            scalar2=None, op0=alu.mult,
                        )
                        for h in (0, 1):
                            pb = pbp.tile([P, H], FP32, tag="pb")
                            nc.tensor.matmul(
                                out=pb, lhsT=dw,
                                rhs=pt[:, j0 + j, h * H:(h + 1) * H],
                                start=True, stop=False,
                            )
                            nc.tensor.matmul(
                                out=pb, lhsT=do,
                                rhs=x_sb[:, g, h * H:(h + 1) * H],
                                start=False, stop=True,
                            )
                            dst = st[:, j, h * H:(h + 1) * H]
                            # GPSIMD cannot read PSUM: drain on ACT + DVE
                            if ncopy % 4 == 3:
                                nc.vector.tensor_copy(out=dst, in_=pb)
                            else:
                                nc.scalar.copy(out=dst, in_=pb)
                            ncopy += 1
                    src = st
                dst = out[ga * P:(ga + 2) * P, :].rearrange(
                    "(g p) d -> p g d", p=P
                )
                nc.sync.dma_start(out=dst, in_=src)

    nc.compile()
    return nc


_NC_CACHE: bass.Bass | None = None


def _get_nc() -> bass.Bass:
    global _NC_CACHE
    if _NC_CACHE is None:
        _NC_CACHE = build_nc()
    return _NC_CACHE


def kernel(x: np.ndarray, processed: np.ndarray, w_router: np.ndarray,
           **run_kwargs) -> np.ndarray:
    from concourse.bass_utils import run_bass_kernel_spmd

    x = np.ascontiguousarray(x, dtype=np.float32)
    processed = np.ascontiguousarray(processed, dtype=np.float32)
    w2d = np.ascontiguousarray(w_router.reshape(1, D), dtype=np.float32)

    nc = _get_nc()
    in_maps = [
        {"x": x[b], "proc": processed[b], "w": w2d} for b in range(B)
    ]
    res = run_bass_kernel_spmd(nc, in_maps, core_ids=list(range(B)),
                               **run_kwargs)
    out = np.stack(
        [np.asarray(res.results[b]["out"]).astype(np.float32)
         for b in range(B)]
    )
    kernel.last_results = res
    return out
